# revision 7
# baseline (speedup 1.0000x reference)
"""Trainium2 Bass kernel for CantorMultiheadFusion — v2.

Reference math:
    h      = x @ W_in^T                        # [B,S,D]
    d[s,k] = distances[s, routes[s,k]]
    w      = softmax(-d, axis=-1)              # [S,K]
    fused  = sum_k w[s,k] * h[:, routes[s,k]]  # [B,S,D]
    out    = fused @ W_out^T + b_out + x

Because fusion weights are shared across the feature dim, the gather commutes
with both projections:
    out = (A @ x) @ (W_out @ W_in)^T + b_out + x
where A[s,j] = C[s,j] * exp(-distances[s,j]) / denom(s),
      C[s,j] = #{k : routes[s,k] == j}   (integer multiplicity),
      denom(s) = sum_j C[s,j] * exp(-distances[s,j]).

v2 packs the routing info into ONE fp8 tensor per core: dd[j,s] = d - ln(C)
(clipped to +448 where C == 0 so exp underflows to +0). ln(C) != 0 only at
duplicate routes (~0.05% of entries), so this is index-marshalling plus a
dtype cast of the distance column block; exp(-dd), the softmax normalization
and all matmuls run on device. Vs v1 this halves the routing wire bytes and
deletes the on-device subtract chain entirely.

Sharding: sequence-parallel over S across 8 cores (256 rows each); x is
replicated since the A@x contraction needs all S rows — except each core's
OWN 256 rows: the host rotates the 16 contraction tiles per core so those
rows sit in tiles 0,1 and the device re-derives them from the bf16 residual
(xres) with a cast instead of pulling them over the wire a second time.
The residual itself is folded into stepB's PSUM accumulation via an identity
matmul against the denominator-scaled residual (resp), so each output chunk
needs only a single rdT scale as its epilogue.

fp8 DoubleRow matmul datapath (0.5 PE cycles/row):
  stepA: t^T[e,s] += x[b,jpair](lhsT) @ numer[jpair](rhs)    (j contract)
  Wc   : Wc[e,i]  = sum_a (16*W_in)[a,e] * W_outT[a,i]       (a contract)
  stepB: out[s,i] += tT[epair](lhsT) @ Wc[epair](rhs)        (e contract)
W_in is pre-scaled by 16 (exact power of 2) so Wc sits in the fp8e4m3 sweet
spot; the factor is folded into the softmax denominator by using 16-valued
ones in the denominator matmul, so the epilogue's rdT multiply removes it.

Wire order (one SP queue, kernel is wire-bound): ld | xres01 xb0a xres23 |
W | xb1a xb0b xb1b | xb1c | ident — the residual halves ride early to feed
the pair-0 reconstruction, b1's x tail shrinks to one DoubleRow pair so the
final stepA->tt->stepB->epilogue chain hangs off as little wire as possible,
the tiny identity tile rides dead last (its consumer is ~1.3us post-wire),
and both batches' outputs ship as one DMA each after the input stream
drains. Output is written bf16 and upcast to f32 on the host.
"""

import os
import sys

import numpy as np

for _p in ("/opt/trn_rl_repo",):
    if os.path.isdir(_p) and _p not in sys.path:
        sys.path.insert(0, _p)


def _ensure_axon_hooks_stub():
    import types
    try:
        import antenv.axon_hooks  # noqa: F401
    except ModuleNotFoundError:
        try:
            import antenv
        except ModuleNotFoundError:
            return
        _stub = types.ModuleType("antenv.axon_hooks")
        _stub.get_axon_ntff_profile_hook = lambda: None
        sys.modules["antenv.axon_hooks"] = _stub
        antenv.axon_hooks = _stub


_ensure_axon_hooks_stub()

B, S, D, K = 2, 2048, 512, 64
N_CORES = 8
SLOC = S // N_CORES          # 256 sequence rows per core
NJ = S // 128                # 16 contraction tiles
NP = NJ // 2                 # 8 DoubleRow contraction pairs
NE = D // 128                # 4 feature chunks
NSC = SLOC // 128            # 2 seq chunks per core

WSCALE = 16.0                # host scale on W_in (exact power of 2)
ONES_VAL = WSCALE            # folded into the softmax denominator

LDC = [(0, 4), (4, 4)]                      # numer stream chunks (pairs)
# x pair groups per batch. Pair 0 (j-tiles 0,1) is NOT shipped: the host
# rotates the contraction tiles per core so tiles 0,1 are the core's own
# rows, which already ship in bf16 as the residual — the device casts them
# to fp8 instead of pulling them over the wire a second time.
XG = {0: [(1, 4), (5, 3)], 1: [(1, 4), (5, 2), (7, 1)]}
SSHIP = S - 2 * 128                          # x rows on the wire per batch

_CACHE = {}
LAST_RESULTS = None


def _build_nc(with_bias=True):
    import concourse.bacc as bacc
    import concourse.mybir as mybir
    import concourse.tile as tile

    F32 = mybir.dt.float32
    BF16 = mybir.dt.bfloat16
    F8 = mybir.dt.float8e4
    MUL = mybir.AluOpType.mult
    ADD = mybir.AluOpType.add
    DR = mybir.MatmulPerfMode.DoubleRow
    EXP = mybir.ActivationFunctionType.Exp
    CPY = mybir.ActivationFunctionType.Copy

    nc = bacc.Bacc("TRN2", target_bir_lowering=False, debug=False, num_devices=1,
                   num_swdge_queues=2)

    x_d = nc.dram_tensor("x", [B, SSHIP, D], F8, kind="ExternalInput").ap()
    ident_d = nc.dram_tensor("ident", [128, 128], BF16, kind="ExternalInput").ap()
    ldp_d = nc.dram_tensor("ldp", [128, NJ * SLOC], F8, kind="ExternalInput").ap()
    xres_d = nc.dram_tensor("xres", [B, SLOC, D], BF16, kind="ExternalInput").ap()
    wpack_d = nc.dram_tensor("wpack", [2, D, D], F8, kind="ExternalInput").ap()
    bout_d = nc.dram_tensor("b_out", [1, D], F32, kind="ExternalInput").ap()
    out_d = nc.dram_tensor("out", [B, SLOC, D], BF16, kind="ExternalOutput").ap()

    with tile.TileContext(nc) as tc:
        with (
            tc.tile_pool(name="big", bufs=1) as big,
            tc.tile_pool(name="ldstream", bufs=2) as ldstream,
            tc.tile_pool(name="pa", bufs=4, space="PSUM") as pa,
            tc.tile_pool(name="pden", bufs=1, space="PSUM") as pdenp,
            tc.tile_pool(name="ptr", bufs=3, space="PSUM") as ptr,
        ):
            # ---- persistent SBUF ----
            xb = {(b, g): big.tile([128, 2 * pn, D], F8, name=f"xb{b}_{g}")
                  for b in range(B) for g, (p0, pn) in enumerate(XG[b])}
            xown = {b: big.tile([128, 2, D], F8, name=f"xown{b}")
                    for b in range(B)}
            numer = big.tile([128, NJ, SLOC], F8)       # [128j, j, 256s]
            w_sb = big.tile([128, 8, D], F8)            # [128a, (w,t), *]
            wcT = big.tile([128, NE, D], F8)            # [128e, ec, 512i]
            ones2 = big.tile([128, 2], F8)
            xres_sb = big.tile([128, B * NSC, D], BF16)
            resp = big.tile([128, B * NSC, D], BF16)    # denom-scaled residual
            ident = big.tile([128, 128], BF16)
            tT = big.tile([128, B * NE, SLOC], F8)      # [128e, (b,ec), 256s]
            outbuf = big.tile([128, B * NSC, D], BF16)
            rdT = big.tile([128, NSC], F32)
            if with_bias:
                bias_sb = big.tile([1, D], F32)
                bias_bc = big.tile([128, D], F32)
                ones_r = big.tile([1, 128], F32)
                resb = big.tile([128, B * NSC, D], F32)

            nc.vector.memset(ones2[:], ONES_VAL)

            # stepA PSUM: bank (b,p) holds ec=2p (cols :SLOC) and ec=2p+1
            # (cols SLOC:). Exactly one start=True per bank region.
            pdT = pdenp.tile([128, NSC], F32)
            pts = {(b, p): pa.tile([128, 2 * SLOC], F32,
                                   name=f"pts{b}_{p}", tag="acc")
                   for b in range(B) for p in range(NE // 2)}

            def pta(b, ec):
                return pts[(b, ec // 2)][:, (ec % 2) * SLOC:(ec % 2 + 1) * SLOC]

            if with_bias:
                nc.scalar.dma_start(out=bias_sb[:1, :], in_=bout_d[:, :])
                nc.vector.memset(ones_r[:], 1.0)
                pb = ptr.tile([128, D], F32, name="pb", tag="tr")
                nc.tensor.matmul(pb[:], lhsT=ones_r[:1, :], rhs=bias_sb[:1, :],
                                 start=True, stop=True)
                nc.vector.tensor_copy(bias_bc[:], pb[:])

            def npair(pp):
                return numer[:, 2 * pp:2 * pp + 2, :]

            def denom_mm(pp):
                n3 = npair(pp)
                for sc in range(NSC):
                    nc.tensor.matmul(
                        pdT[:, sc:sc + 1],
                        lhsT=n3[:, :, sc * 128:(sc + 1) * 128],
                        rhs=ones2[:].rearrange("p (j o) -> p j o", j=2),
                        start=(pp == 0 and sc == 0),
                        stop=(pp == NP - 1 and sc == NSC - 1),
                        perf_mode=DR, skip_group_check=True)

            def stepa(pp, b):
                if pp == 0:
                    x3 = xown[b][:, 0:2, :]
                else:
                    for g, (p0, pn) in enumerate(XG[b]):
                        if p0 <= pp < p0 + pn:
                            x3 = xb[(b, g)][:,
                                            2 * (pp - p0):2 * (pp - p0) + 2, :]
                            break
                n3 = npair(pp)
                for ec in range(NE):
                    nc.tensor.matmul(
                        pta(b, ec),
                        lhsT=x3[:, :, ec * 128:(ec + 1) * 128],
                        rhs=n3,
                        start=(pp == 0 and ec % 2 == 0),
                        stop=(pp == NP - 1 and ec % 2 == 1),
                        perf_mode=DR, skip_group_check=True)

            def x_dma(b, g):
                xp0, xpn = XG[b][g]
                r0 = 2 * (xp0 - 1) * 128     # shipped rows start at pair 1
                nc.sync.dma_start(
                    out=xb[(b, g)][:],
                    in_=x_d[b, r0:r0 + 2 * xpn * 128, :]
                        .rearrange("(j p) e -> p j e", p=128))

            def stepb_mm(b, sc, with_ident=True):
                # po[s,i] = sum_e t[e,s] Wc[e,i]  (+  denom[s]*res[s,i])
                # (the residual rides into the PSUM group via an identity
                # matmul, so the epilogue is a single rdT scale; the final
                # out1-critical chunk skips the ident mm — its 213ns sits on
                # the PE spine — and fuses the residual in the epilogue)
                po = pa.tile([128, D], F32, name=f"po{b}_{sc}", tag="acc")
                t3 = tT[:, b * NE:(b + 1) * NE, :]
                for ep in range(2):
                    nc.tensor.matmul(
                        po[:],
                        lhsT=t3[:, 2 * ep:2 * ep + 2, sc * 128:(sc + 1) * 128],
                        rhs=wcT[:, 2 * ep:2 * ep + 2, :],
                        start=(ep == 0),
                        stop=(not with_ident and ep == 1),
                        perf_mode=DR)
                if with_ident:
                    nc.tensor.matmul(
                        po[:], lhsT=ident[:],
                        rhs=resp[:, b * NSC + sc, :],
                        start=False, stop=True)
                return po

            def resp_prep(b, sc):
                # denom-scaled residual: resp = res * denom (per-partition)
                i = b * NSC + sc
                res = resb if with_bias else xres_sb
                nc.vector.tensor_scalar(
                    out=resp[:, i, :], in0=res[:, i, :],
                    scalar1=pdT[:, sc:sc + 1], scalar2=None, op0=MUL)

            def epi_scale(b, sc, po, eng="act"):
                i = b * NSC + sc
                if eng == "act":
                    nc.scalar.activation(outbuf[:, i, :], po[:], CPY,
                                         scale=rdT[:, sc:sc + 1])
                else:
                    nc.vector.tensor_scalar(
                        out=outbuf[:, i, :], in0=po[:],
                        scalar1=rdT[:, sc:sc + 1], scalar2=None, op0=MUL)

            # ---- input stream: one SP queue, explicit wire order ----
            ld_ts = []
            for c, (p0, pn) in enumerate(LDC):
                ld_t = ldstream.tile([128, 2 * pn, SLOC], F8,
                                     name=f"ld{c}", tag="ld")
                ld_ts.append(ld_t)
                nc.sync.dma_start(
                    out=ld_t[:],
                    in_=ldp_d[:, 2 * p0 * SLOC:2 * (p0 + pn) * SLOC]
                        .rearrange("p (j c) -> p j c", j=2 * pn))
            xres_flat = xres_d.rearrange("b (sc p) e -> p (b sc) e", p=128)
            nc.sync.dma_start(out=xres_sb[:, 0:NSC, :],
                              in_=xres_flat[:, 0:NSC, :])
            x_dma(0, 0)
            nc.sync.dma_start(out=xres_sb[:, NSC:2 * NSC, :],
                              in_=xres_flat[:, NSC:2 * NSC, :])
            nc.sync.dma_start(
                out=w_sb[:],
                in_=wpack_d.rearrange("w (t p) e -> p (w t) e", p=128))
            x_dma(1, 0)
            x_dma(0, 1)
            x_dma(1, 1)
            x_dma(1, 2)  # 1 pair: the shortest possible post-wire stepA tail
            # ident rides last: its only consumer is the stepB residual mm
            # (~1.3us after wire end), so it never delays the x stream
            nc.sync.dma_start(out=ident[:], in_=ident_d[:, :])

            # ---- ACT: exp directly on the packed fp8 stream ----
            for c, (p0, pn) in enumerate(LDC):
                nc.scalar.activation(numer[:, 2 * p0:2 * (p0 + pn), :],
                                     ld_ts[c][:], EXP, scale=-1.0)

            # pair-0 x tiles: cast the bf16 residual rows to fp8 (DVE is idle
            # this early; saves re-shipping the core's own rows as fp8)
            for b in range(B):
                nc.vector.tensor_copy(xown[b][:],
                                      xres_sb[:, b * NSC:(b + 1) * NSC, :])

            if with_bias:
                for b in range(B):
                    for sc in range(NSC):
                        i = b * NSC + sc
                        nc.vector.tensor_add(resb[:, i, :],
                                             xres_sb[:, i, :], bias_bc[:])

            # ---- PE program in data-arrival order ----
            for pp in range(0, 4):
                denom_mm(pp)
                stepa(pp, 0)
            stepa(4, 0)
            for pp in range(0, 5):
                stepa(pp, 1)
            for pp in range(4, NP):
                denom_mm(pp)
            nc.vector.reciprocal(rdT[:], pdT[:])
            for b in range(B):
                for sc in range(NSC):
                    resp_prep(b, sc)

            # Wc[e,i] = sum_a (16*W_in)[a,e] * W_outT[a,i], fp8 DoubleRow
            for ec in range(NE):
                pw = ptr.tile([128, D], F32, name=f"pw{ec}", tag="tr")
                for ap_ in range(2):
                    nc.tensor.matmul(
                        pw[:],
                        lhsT=w_sb[:, 2 * ap_:2 * ap_ + 2,
                                  ec * 128:(ec + 1) * 128],
                        rhs=w_sb[:, 4 + 2 * ap_:4 + 2 * ap_ + 2, :],
                        start=(ap_ == 0), stop=(ap_ == 1),
                        perf_mode=DR)
                if ec < 2:
                    nc.vector.tensor_copy(wcT[:, ec, :], pw[:])
                else:
                    nc.scalar.copy(wcT[:, ec, :], pw[:])

            for pp in range(5, NP):
                stepa(pp, 0)
            # tt(b0): PSUM->SBUF fp8, one bank on DVE, one on ACT
            nc.vector.tensor_copy(tT[:, 0:2, :], pts[(0, 0)][:])
            nc.scalar.copy(tT[:, 2:4, :], pts[(0, 1)][:])

            for pp in range(5, 7):
                stepa(pp, 1)
            stepa(7, 1)
            nc.vector.tensor_copy(tT[:, NE:NE + 2, :], pts[(1, 0)][:])
            nc.scalar.copy(tT[:, NE + 2:NE + 4, :], pts[(1, 1)][:])

            # the ident-free group first: its epilogue (DVE stt) is the
            # longer chain, so its po must land earliest
            po01 = stepb_mm(0, 1, with_ident=False)
            po00 = stepb_mm(0, 0)
            epi_scale(0, 0, po00, eng="act")
            res_t0 = resb if with_bias else xres_sb
            nc.vector.scalar_tensor_tensor(
                out=outbuf[:, 1, :], in0=po01[:],
                scalar=rdT[:, 1:2], in1=res_t0[:, 1, :],
                op0=MUL, op1=ADD)
            nc.sync.dma_start(
                out=out_d[0].rearrange("(sc p) e -> p sc e", p=128),
                in_=outbuf[:, 0:NSC, :])
            po11 = stepb_mm(1, 1, with_ident=False)
            po10 = stepb_mm(1, 0)
            epi_scale(1, 0, po10, eng="act")
            res_t = resb if with_bias else xres_sb
            nc.vector.scalar_tensor_tensor(
                out=outbuf[:, NSC + 1, :], in0=po11[:],
                scalar=rdT[:, 1:2], in1=res_t[:, NSC + 1, :],
                op0=MUL, op1=ADD)
            nc.sync.dma_start(
                out=out_d[1].rearrange("(sc p) e -> p sc e", p=128),
                in_=outbuf[:, NSC:2 * NSC, :])

    nc.compile()
    return nc


def _get_nc(with_bias=True):
    key = ("nc", with_bias)
    if key not in _CACHE:
        _CACHE[key] = _build_nc(with_bias)
    return _CACHE[key]


def prep_in_maps(x, routes, distances, W_in, W_out, b_out):
    """Host-side sharding/marshalling: per-core input dicts."""
    import ml_dtypes
    import concourse.mybir as mybir

    bf16 = ml_dtypes.bfloat16
    f8 = mybir.dt.np(mybir.dt.float8e4)
    x = np.ascontiguousarray(np.asarray(x, dtype=np.float32))
    routes = np.asarray(routes, dtype=np.int32)
    distances = np.ascontiguousarray(np.asarray(distances, dtype=np.float32))
    b_out = np.ascontiguousarray(np.asarray(b_out, dtype=np.float32)).reshape(1, D)

    wpack = np.empty((2, D, D), dtype=np.float32)
    wpack[0] = np.asarray(W_in, dtype=np.float32) * WSCALE
    wpack[1] = np.asarray(W_out, dtype=np.float32).T
    wpack = wpack.astype(f8)

    x_8 = x.astype(f8)
    xres_b = x.astype(bf16)

    # Route multiplicity C^T[j, s] folded into the distance block as
    # dd = d - ln(C): identical to the distance for the ~unique routes,
    # shifted at the rare duplicates, +448 (fp8 max) where C == 0 so the
    # device-side exp underflows to +0. Index marshalling + dtype cast.
    flat = routes.astype(np.int64).ravel() * S + np.repeat(np.arange(S, dtype=np.int64), K)
    countsT = np.bincount(flat, minlength=S * S).reshape(S, S)
    dd = distances.T.copy()
    dup = countsT > 1
    dd[dup] -= np.log(countsT[dup].astype(np.float32))
    dd[countsT == 0] = 448.0
    dd = np.minimum(dd, 448.0, dtype=np.float32)

    x8t = x_8.reshape(B, NJ, 128, D)
    in_maps = []
    for c in range(N_CORES):
        sl = slice(c * SLOC, (c + 1) * SLOC)
        # Rotate the contraction tiles so tiles 0,1 are this core's own rows
        # (reconstructed on device from the bf16 residual); ship tiles 2..15.
        order = [(j + 2 * c) % NJ for j in range(NJ)]
        xship = np.ascontiguousarray(
            x8t[:, order[2:], :, :]).reshape(B, SSHIP, D)
        ddc = dd[:, sl].reshape(NJ, 128, SLOC)[order]
        ldp = np.ascontiguousarray(
            ddc.transpose(1, 0, 2)).reshape(128, NJ * SLOC)
        in_maps.append({
            "x": xship,
            "ldp": ldp.astype(f8),
            "xres": np.ascontiguousarray(xres_b[:, sl, :]),
            "wpack": wpack,
            "ident": np.eye(128, dtype=np.float32).astype(bf16),
            "b_out": b_out,
        })
    return in_maps


def kernel(x, routes, distances, W_in, W_out, b_out):
    global LAST_RESULTS
    from concourse import bass_utils

    in_maps = prep_in_maps(x, routes, distances, W_in, W_out, b_out)
    with_bias = bool(np.any(np.asarray(b_out)))
    nc = _get_nc(with_bias)
    _CACHE["last_nc"] = nc
    res = bass_utils.run_bass_kernel_spmd(nc, in_maps, core_ids=list(range(N_CORES)))
    LAST_RESULTS = res
    out = np.concatenate(
        [res.results[c]["out"].astype(np.float32) for c in range(N_CORES)],
        axis=1)
    return out


if __name__ == "__main__":
    rng = np.random.default_rng(0)
    inputs = {
        "x": rng.standard_normal((B, S, D), dtype=np.float32),
        "routes": rng.integers(0, S, (S, K)).astype(np.int32),
        "distances": rng.random((S, S), dtype=np.float32),
        "W_in": (rng.standard_normal((D, D), dtype=np.float32) / np.sqrt(D)).astype(np.float32),
        "W_out": (rng.standard_normal((D, D), dtype=np.float32) / np.sqrt(D)).astype(np.float32),
        "b_out": np.zeros(D, dtype=np.float32),
    }
    out = kernel(**inputs)
    print("out", out.shape, out.dtype)
